# revision 7
# baseline (speedup 1.0000x reference)
"""Distributed Trainium2 Bass kernel for 3-layer GraphSAGE (nn_GCCN_4492535791673).

Strategy (8 NeuronCores):
- Nodes block-partitioned across cores (12500/core, padded to 12544 = 98*128).
- Per layer, each core holds the full projected-feature table P = h @ Wl
  (bf16, replicated into core-local DRAM via AllGather).
- Edges partitioned by destination core; per core the edge stream is grouped
  into 128-edge columns per destination node tile. Source rows are fetched by
  indirect DMA (128 rows per instruction) from the local replica.
- Aggregation: per column, a one-hot selector (built on DVE from host-provided
  dst offsets via iota/is_equal) is the stationary matmul operand; gathered
  rows are the moving operand; PSUM accumulates segment sums per 128-node tile.
- mean-scale (1/cnt) is a per-partition ACT scale; h@Wr + bias via PE;
  LayerNorm via bn_stats/bn_aggr + fused scalar ops; relu+residual on DVE.
- Output projection accumulated per layer into SBUF, one DMA out at the end.
"""
import numpy as np
import ml_dtypes

import concourse.bass as bass
import concourse.bacc as bacc
import concourse.tile as tile
from concourse import mybir
from concourse.bass_utils import run_bass_kernel_spmd

# Problem constants (hardcoded per contest contract)
N, F_IN, H, E, L, C = 100000, 128, 128, 1600000, 3, 16
LN_EPS = 1e-5
M = 8                     # cores
NS = N // M               # 12500 real nodes per core
NT = (NS + 127) // 128    # 98 tiles
NSP = NT * 128            # 12544 padded rows per core
V = M * NSP               # 100352 table rows
ZROW = NS                 # a zeroed pad row (core 0 block), global id = NS
BF16 = mybir.dt.bfloat16
F32 = mybir.dt.float32
I32 = mybir.dt.int32

_COMPILE_CACHE = {}


# ----------------------------------------------------------------- host prep

def _derived():
    """Recompute derived constants (so tests can shrink the problem)."""
    global NS, NT, NSP, V, ZROW
    NS = N // M
    NT = (NS + 127) // 128
    NSP = NT * 128
    V = M * NSP
    ZROW = NS


def _preprocess(edge_index):
    src = np.asarray(edge_index[0]).astype(np.int64)
    dst = np.asarray(edge_index[1]).astype(np.int64)
    deg = np.bincount(dst, minlength=N).astype(np.float64)
    inv_cnt_full = (1.0 / np.maximum(deg, 1.0)).astype(np.float32)

    src_core = src // NS
    src_pg = src_core * NSP + (src - src_core * NS)   # padded global id
    dst_core = dst // NS

    per_core = []
    m_counts = np.zeros((M, NT), np.int64)
    for c in range(M):
        sel = np.nonzero(dst_core == c)[0]
        spg = src_pg[sel]
        r = dst[sel] - c * NS                          # local rank 0..NS-1
        t = r // 128
        order = np.argsort(t, kind="stable")
        spg, r, t = spg[order], r[order], t[order]
        doffv = (r % 128).astype(np.int64)
        np.add.at(m_counts[c], t, 1)
        per_core.append((spg, doffv, t))

    K = np.ceil(m_counts.max(axis=0) / 128.0).astype(np.int64)  # [NT] cols
    K = np.maximum(K, 1)
    col_start = np.zeros(NT, np.int64)
    col_start[1:] = np.cumsum(K)[:-1]
    totc = int(K.sum())

    idx_np, doff_np, invc_np = [], [], []
    for c in range(M):
        spg, doffv, t_arr = per_core[c]
        cols_src = np.full((totc, 128), ZROW, np.int64)
        cols_dof = np.zeros((totc, 128), np.int64)
        uniq, first, counts = np.unique(t_arr, return_index=True,
                                        return_counts=True)
        for u, f, cnt in zip(uniq, first, counts):
            tt = int(u)
            cb = col_start[tt]
            ncol = int(K[tt])
            buf_s = np.full(ncol * 128, ZROW, np.int64)
            buf_d = np.zeros(ncol * 128, np.int64)
            buf_s[:cnt] = spg[f:f + cnt]
            buf_d[:cnt] = doffv[f:f + cnt]
            cols_src[cb:cb + ncol] = buf_s.reshape(ncol, 128)
            cols_dof[cb:cb + ncol] = buf_d.reshape(ncol, 128)
        idx_np.append(cols_src.T.astype(np.int32).copy())          # [128, totc]
        doff_np.append(cols_dof.T.astype(ml_dtypes.bfloat16).copy())
        icc = np.ones(NSP, np.float32)
        icc[:NS] = inv_cnt_full[c * NS:(c + 1) * NS]
        invc_np.append(icc.reshape(NT, 128).T.copy())

    plan = dict(K=K, col_start=col_start, totc=totc)
    return plan, idx_np, doff_np, invc_np


# ------------------------------------------------------------------- builder

def _build(plan):
    K = plan["K"]
    col_start = plan["col_start"]
    totc = plan["totc"]

    nc = bacc.Bacc("TRN2", target_bir_lowering=False, debug=False, num_devices=M)

    x_in = nc.dram_tensor("x", [NSP, F_IN], F32, kind="ExternalInput")
    idx_in = nc.dram_tensor("idx", [128, totc], I32, kind="ExternalInput")
    doff_in = nc.dram_tensor("doff", [128, totc], BF16, kind="ExternalInput")
    invc_in = nc.dram_tensor("invc", [128, NT], F32, kind="ExternalInput")
    win_in = nc.dram_tensor("win", [H, H], BF16, kind="ExternalInput")
    wl_in = nc.dram_tensor("wl", [H, L * H], BF16, kind="ExternalInput")
    wr_in = nc.dram_tensor("wr", [H, L * H], BF16, kind="ExternalInput")
    wo_in = nc.dram_tensor("wo", [H, L * C], BF16, kind="ExternalInput")
    brow_in = nc.dram_tensor("brow", [1, H + L * H + C], BF16, kind="ExternalInput")
    g_in = nc.dram_tensor("gt", [128, L * H], F32, kind="ExternalInput")
    b_in_t = nc.dram_tensor("bt", [128, L * H], F32, kind="ExternalInput")
    ones_in = nc.dram_tensor("ones1", [1, 128], BF16, kind="ExternalInput")
    iota_in = nc.dram_tensor("iotab", [128, 128], BF16, kind="ExternalInput")
    ident_in = nc.dram_tensor("identb", [128, 128], BF16, kind="ExternalInput")
    pmask_in = nc.dram_tensor("pmask", [128, 1], F32, kind="ExternalInput")
    out_ext = nc.dram_tensor("out", [NSP, C], F32, kind="ExternalOutput")

    p_stage = [nc.dram_tensor(f"p_stage{i}", [NSP, H], BF16) for i in range(2)]
    p_rep = [nc.dram_tensor(f"p_rep{i}", [V, H], BF16, addr_space="Shared")
             for i in range(2)]

    with tile.TileContext(nc) as tc:
        from contextlib import ExitStack
        ctx = ExitStack()
        sbp = ctx.enter_context(tc.tile_pool(name="persist", bufs=1))
        sbw = ctx.enter_context(tc.tile_pool(name="work", bufs=3))
        sbg = ctx.enter_context(tc.tile_pool(name="gather", bufs=10))
        psA = ctx.enter_context(tc.tile_pool(name="psA", bufs=2, space="PSUM"))
        psB = ctx.enter_context(tc.tile_pool(name="psB", bufs=2, space="PSUM"))
        psT = ctx.enter_context(tc.tile_pool(name="psT", bufs=2, space="PSUM"))
        psO = ctx.enter_context(tc.tile_pool(name="psO", bufs=1, space="PSUM"))

        idx_sb = sbp.tile([128, totc], I32)
        nc.sync.dma_start(out=idx_sb[:], in_=idx_in[:])
        doff_sb = sbp.tile([128, totc], BF16)
        nc.sync.dma_start(out=doff_sb[:], in_=doff_in[:])
        invc_sb = sbp.tile([128, NT], F32)
        nc.sync.dma_start(out=invc_sb[:], in_=invc_in[:])
        win_sb = sbp.tile([H, H], BF16)
        nc.sync.dma_start(out=win_sb[:], in_=win_in[:])
        wl_sb = sbp.tile([H, L * H], BF16)
        nc.sync.dma_start(out=wl_sb[:], in_=wl_in[:])
        wr_sb = sbp.tile([H, L * H], BF16)
        nc.sync.dma_start(out=wr_sb[:], in_=wr_in[:])
        wo_sb = sbp.tile([H, L * C], BF16)
        nc.sync.dma_start(out=wo_sb[:], in_=wo_in[:])
        brow_sb = sbp.tile([1, H + L * H + C], BF16)
        nc.sync.dma_start(out=brow_sb[:], in_=brow_in[:])
        gt_sb = sbp.tile([128, L * H], F32)
        nc.sync.dma_start(out=gt_sb[:], in_=g_in[:])
        bt_sb = sbp.tile([128, L * H], F32)
        nc.sync.dma_start(out=bt_sb[:], in_=b_in_t[:])
        ones_sb = sbp.tile([1, 128], BF16)
        nc.sync.dma_start(out=ones_sb[:], in_=ones_in[:])
        iota_sb = sbp.tile([128, 128], BF16)
        nc.sync.dma_start(out=iota_sb[:], in_=iota_in[:])
        ident_sb = sbp.tile([128, 128], BF16)
        nc.sync.dma_start(out=ident_sb[:], in_=ident_in[:])
        pmask_sb = sbp.tile([128, 1], F32)
        nc.sync.dma_start(out=pmask_sb[:], in_=pmask_in[:])

        hA = sbp.tile([128, NT * H], BF16)
        hB = sbp.tile([128, NT * H], BF16)
        hT = sbp.tile([128, NT * H], BF16)
        out_acc = sbp.tile([128, NT * C], F32)

        PAD0 = NS - (NT - 1) * 128   # first pad partition in the last tile

        def stage_p(pP, layer_next, t):
            pst = sbw.tile([128, H], BF16, tag="pst")
            if t == NT - 1 and PAD0 < 128:
                nc.scalar.activation(out=pst[:], in_=pP[:],
                                     func=mybir.ActivationFunctionType.Copy,
                                     scale=pmask_sb[:])
            else:
                nc.scalar.copy(out=pst[:], in_=pP[:])
            nc.sync.dma_start(
                out=p_stage[layer_next % 2][t * 128:(t + 1) * 128, :],
                in_=pst[:])

        # ---- stage 0: input projection + p0 staging
        for t in range(NT):
            xt = sbw.tile([128, F_IN], F32, tag="xt")
            nc.sync.dma_start(out=xt[:], in_=x_in[t * 128:(t + 1) * 128, :])
            xbf = sbw.tile([128, F_IN], BF16, tag="xbf")
            nc.vector.tensor_copy(out=xbf[:], in_=xt[:])
            pT = psT.tile([128, 128], BF16, space="PSUM", tag="pT")
            nc.tensor.transpose(out=pT[:], in_=xbf[:], identity=ident_sb[:])
            xTbf = sbw.tile([128, 128], BF16, tag="xTbf")
            nc.scalar.copy(out=xTbf[:], in_=pT[:])
            pH = psB.tile([128, H], F32, space="PSUM", tag="pB")
            nc.tensor.matmul(out=pH[:], lhsT=ones_sb[:], rhs=brow_sb[:, 0:H],
                             start=True, stop=False)
            nc.tensor.matmul(out=pH[:], lhsT=xTbf[:], rhs=win_sb[:],
                             start=False, stop=True)
            nc.scalar.activation(out=hA[:, t * H:(t + 1) * H], in_=pH[:],
                                 func=mybir.ActivationFunctionType.Relu)
            pT2 = psT.tile([128, 128], BF16, space="PSUM", tag="pT")
            nc.tensor.transpose(out=pT2[:], in_=hA[:, t * H:(t + 1) * H],
                                identity=ident_sb[:])
            nc.scalar.copy(out=hT[:, t * 128:(t + 1) * 128], in_=pT2[:])
            pP = psO.tile([128, H], F32, space="PSUM", tag="pO")
            nc.tensor.matmul(out=pP[:], lhsT=hT[:, t * 128:(t + 1) * 128],
                             rhs=wl_sb[:, 0:H], start=True, stop=True)
            stage_p(pP, 0, t)

        nc.gpsimd.collective_compute(
            "AllGather", mybir.AluOpType.bypass,
            replica_groups=[list(range(M))],
            ins=[p_stage[0].ap().opt()],
            outs=[p_rep[0].ap().opt()],
        )

        # ---- layers
        for layer in range(L):
            h_in = hA if layer % 2 == 0 else hB
            h_out = hB if layer % 2 == 0 else hA
            rep = p_rep[layer % 2]

            for t in range(NT):
                kt = int(K[t])
                cb = int(col_start[t])
                pA = psA.tile([128, H], F32, space="PSUM", tag="pA")
                # gather + one-hot matmul accumulate
                for k0 in range(0, kt, 4):
                    kk = min(4, kt - k0)
                    sel = sbw.tile([128, 4, 128], BF16, tag="sel")
                    nc.vector.tensor_tensor(
                        out=sel[:, 0:kk, :],
                        in0=doff_sb[:, cb + k0:cb + k0 + kk].unsqueeze(-1)
                            .to_broadcast([128, kk, 128]),
                        in1=iota_sb[:].unsqueeze(1)
                            .to_broadcast([128, kk, 128]),
                        op=mybir.AluOpType.is_equal,
                    )
                    for k in range(kk):
                        gcol = sbg.tile([128, H], BF16, tag="gcol")
                        nc.gpsimd.indirect_dma_start(
                            out=gcol[:],
                            out_offset=None,
                            in_=rep[:],
                            in_offset=bass.IndirectOffsetOnAxis(
                                ap=idx_sb[:, cb + k0 + k:cb + k0 + k + 1],
                                axis=0),
                        )
                        nc.tensor.matmul(
                            out=pA[:], lhsT=sel[:, k, :], rhs=gcol[:],
                            start=(k0 + k == 0), stop=(k0 + k == kt - 1),
                        )
                # z = psumA * invc + (bl + h_in @ Wr)
                pB = psB.tile([128, H], F32, space="PSUM", tag="pB")
                nc.tensor.matmul(out=pB[:], lhsT=ones_sb[:],
                                 rhs=brow_sb[:, H + layer * H:H + (layer + 1) * H],
                                 start=True, stop=False)
                nc.tensor.matmul(out=pB[:],
                                 lhsT=hT[:, t * 128:(t + 1) * 128],
                                 rhs=wr_sb[:, layer * H:(layer + 1) * H],
                                 start=False, stop=True)
                zt = sbw.tile([128, H], F32, tag="zt")
                nc.scalar.activation(out=zt[:], in_=pA[:],
                                     func=mybir.ActivationFunctionType.Copy,
                                     scale=invc_sb[:, t:t + 1])
                z = sbw.tile([128, H], F32, tag="z")
                nc.vector.tensor_tensor(out=z[:], in0=zt[:], in1=pB[:],
                                        op=mybir.AluOpType.add)
                st6 = sbw.tile([128, 6], F32, tag="st6")
                nc.vector.bn_stats(out=st6[:], in_=z[:])
                mv = sbw.tile([128, 2], F32, tag="mv")
                nc.vector.bn_aggr(out=mv[:], in_=st6[:])
                veps = sbw.tile([128, 1], F32, tag="veps")
                nc.vector.tensor_scalar(out=veps[:], in0=mv[:, 1:2],
                                        scalar1=LN_EPS, scalar2=None,
                                        op0=mybir.AluOpType.add)
                stdv = sbw.tile([128, 1], F32, tag="stdv")
                nc.scalar.sqrt(out=stdv[:], in_=veps[:])
                rstd = sbw.tile([128, 1], F32, tag="rstd")
                nc.vector.reciprocal(out=rstd[:], in_=stdv[:])
                nmr = sbw.tile([128, 1], F32, tag="nmr")
                nc.vector.tensor_scalar(out=nmr[:], in0=mv[:, 0:1],
                                        scalar1=rstd[:], scalar2=-1.0,
                                        op0=mybir.AluOpType.mult,
                                        op1=mybir.AluOpType.mult)
                u = sbw.tile([128, H], F32, tag="u")
                nc.vector.tensor_scalar(out=u[:], in0=z[:],
                                        scalar1=rstd[:], scalar2=nmr[:],
                                        op0=mybir.AluOpType.mult,
                                        op1=mybir.AluOpType.add)
                v = sbw.tile([128, H], F32, tag="v")
                nc.vector.tensor_tensor(
                    out=v[:], in0=u[:],
                    in1=gt_sb[:, layer * H:(layer + 1) * H],
                    op=mybir.AluOpType.mult)
                nc.vector.tensor_tensor(
                    out=v[:], in0=v[:],
                    in1=bt_sb[:, layer * H:(layer + 1) * H],
                    op=mybir.AluOpType.add)
                rv = sbw.tile([128, H], BF16, tag="rv")
                nc.scalar.activation(out=rv[:], in_=v[:],
                                     func=mybir.ActivationFunctionType.Relu)
                nc.vector.tensor_tensor(out=h_out[:, t * H:(t + 1) * H],
                                        in0=rv[:],
                                        in1=h_in[:, t * H:(t + 1) * H],
                                        op=mybir.AluOpType.add)
                pT = psT.tile([128, 128], BF16, space="PSUM", tag="pT")
                nc.tensor.transpose(out=pT[:],
                                    in_=h_out[:, t * H:(t + 1) * H],
                                    identity=ident_sb[:])
                nc.scalar.copy(out=hT[:, t * 128:(t + 1) * 128], in_=pT[:])
                pO = psO.tile([128, C], F32, space="PSUM", tag="pO")
                if layer == 0:
                    nc.tensor.matmul(out=pO[:], lhsT=ones_sb[:],
                                     rhs=brow_sb[:, H + L * H:H + L * H + C],
                                     start=True, stop=False)
                    nc.tensor.matmul(out=pO[:],
                                     lhsT=hT[:, t * 128:(t + 1) * 128],
                                     rhs=wo_sb[:, 0:C],
                                     start=False, stop=True)
                    nc.vector.tensor_copy(out=out_acc[:, t * C:(t + 1) * C],
                                          in_=pO[:])
                else:
                    nc.tensor.matmul(
                        out=pO[:], lhsT=hT[:, t * 128:(t + 1) * 128],
                        rhs=wo_sb[:, layer * C:(layer + 1) * C],
                        start=True, stop=True)
                    nc.vector.tensor_tensor(
                        out=out_acc[:, t * C:(t + 1) * C],
                        in0=out_acc[:, t * C:(t + 1) * C],
                        in1=pO[:], op=mybir.AluOpType.add)
                if layer < L - 1:
                    pP = psO.tile([128, H], F32, space="PSUM", tag="pO")
                    nc.tensor.matmul(
                        out=pP[:], lhsT=hT[:, t * 128:(t + 1) * 128],
                        rhs=wl_sb[:, (layer + 1) * H:(layer + 2) * H],
                        start=True, stop=True)
                    stage_p(pP, layer + 1, t)

            if layer < L - 1:
                nc.gpsimd.collective_compute(
                    "AllGather", mybir.AluOpType.bypass,
                    replica_groups=[list(range(M))],
                    ins=[p_stage[(layer + 1) % 2].ap().opt()],
                    outs=[p_rep[(layer + 1) % 2].ap().opt()],
                )

        nc.sync.dma_start(
            out=out_ext[:].rearrange('(t p) c -> p t c', p=128),
            in_=out_acc[:].rearrange('p (t c) -> p t c', c=C))

        ctx.close()

    nc.compile()
    return nc


# -------------------------------------------------------------------- driver

def _make_in_maps(x, idx_np, doff_np, invc_np, params):
    W_in, b_in, Wl, bl, Wr, ln_g, ln_b, W_out, b_out = params
    bf = ml_dtypes.bfloat16
    win = W_in.astype(bf)
    wl = np.concatenate([Wl[l] for l in range(L)], axis=1).astype(bf)
    wr = np.concatenate([Wr[l] for l in range(L)], axis=1).astype(bf)
    wo = np.concatenate([W_out[l * H:(l + 1) * H] for l in range(L)],
                        axis=1).astype(bf)
    brow = np.concatenate([b_in, bl.reshape(-1), b_out])[None, :].astype(bf)
    gt = np.concatenate([np.tile(ln_g[l], (128, 1)) for l in range(L)],
                        axis=1).astype(np.float32)
    bt = np.concatenate([np.tile(ln_b[l], (128, 1)) for l in range(L)],
                        axis=1).astype(np.float32)
    ones1 = np.ones((1, 128), bf)
    pad0 = NS - (NT - 1) * 128
    pmask = (np.arange(128) < pad0).astype(np.float32)[:, None]
    iotab = np.tile(np.arange(128, dtype=np.float32), (128, 1)).astype(bf)
    identb = np.eye(128, dtype=np.float32).astype(bf)

    in_maps = []
    for c in range(M):
        xs = np.zeros((NSP, F_IN), np.float32)
        xs[:NS] = x[c * NS:(c + 1) * NS]
        in_maps.append({
            "x": xs, "idx": idx_np[c], "doff": doff_np[c], "invc": invc_np[c],
            "win": win, "wl": wl, "wr": wr, "wo": wo, "brow": brow,
            "gt": gt, "bt": bt, "ones1": ones1, "iotab": iotab,
            "identb": identb, "pmask": pmask,
        })
    return in_maps


def kernel(x, edge_index, W_in, b_in, Wl, bl, Wr, ln_g, ln_b, W_out, b_out):
    x = np.asarray(x, np.float32)
    params = tuple(np.asarray(a, np.float32) for a in
                   (W_in, b_in, Wl, bl, Wr, ln_g, ln_b, W_out, b_out))
    ekey = hash(np.asarray(edge_index).tobytes())
    if ekey not in _COMPILE_CACHE:
        plan, idx_np, doff_np, invc_np = _preprocess(edge_index)
        nc = _build(plan)
        _COMPILE_CACHE[ekey] = (nc, idx_np, doff_np, invc_np)
    nc, idx_np, doff_np, invc_np = _COMPILE_CACHE[ekey]
    in_maps = _make_in_maps(x, idx_np, doff_np, invc_np, params)
    res = run_bass_kernel_spmd(nc, in_maps, core_ids=list(range(M)))
    out = np.empty((N, C), np.float32)
    for c in range(M):
        out[c * NS:(c + 1) * NS] = res.results[c]["out"][:NS]
    return out
